# revision 30
# baseline (speedup 1.0000x reference)
"""ALIGNNConv distributed Bass kernel for 8 TRN2 NeuronCores.

Sharding (per spec hint): nodes of g partitioned into 8 contiguous ranges;
g-edges assigned to the owner of their dst node; line-graph edges assigned to
the owner of their dst g-edge. Per-core edge streams are sorted by
(src 32000-row bucket, dst) and padded so all 8 cores share one static SPMD
program:
  - src-row gathers: dma_gather, per-call-rebased int16 indices;
  - dst-row gathers: dma_gather with sorted indices from a per-core table;
  - segment_sum: dedup'd dma_scatter_add (duplicates pre-combined on device
    via a selection-matrix matmul; host points non-first duplicates at a
    trash row).
BatchNorm (training mode) is two-pass with a bf16 HBM bounce of pre-BN
messages and an AllReduce of per-feature statistics. Linear biases cancel
inside BatchNorm and are skipped. sigmoid/softplus are built from Exp/Ln so a
single ACT table serves the kernel; 1/sqrt uses exp(-0.5*ln(x)).
"""
import sys
sys.path.insert(0, "/opt/trn_rl_repo")

import numpy as np
import ml_dtypes

import concourse.bass as bass
import concourse.mybir as mybir
import concourse.tile as tile
from concourse import bacc, bass_utils
from concourse.masks import make_identity

# Route every ACT function through the one table that contains Exp+Ln+Square
# (natural_log_exp_and_others); otherwise the table-load pass inserts a
# ~1.3us table reload before nearly every activation (Exp and Ln first-match
# different tables). Emptying the other tables keeps act_func_set_id indices
# valid while forcing a single resident table.
from concourse import hw_specs as _hw_specs
_orig_get_tables = _hw_specs.get_activation_tables

def _single_table(arch):
    t = _orig_get_tables(arch)
    keep = "natural_log_exp_and_others"
    assert keep in t
    return {name: (fns if name == keep else set()) for name, fns in t.items()}

bacc.get_activation_tables = _single_table

F32 = mybir.dt.float32
BF16 = mybir.dt.bfloat16
I16 = mybir.dt.int16
AF = mybir.ActivationFunctionType
OP = mybir.AluOpType

NC = 8
P = 128
SPAN = 32000          # int16-safe index window (multiple of 128)
CALL_TILES = 16       # tiles of 128 edges per gather/scatter call
NF = 64
EPS = 1e-5

CFG = dict(n_nodes=50000, n_edges=600000, n_lg_edges=1200000)


# =============================================================== host prep

def _bf16(a):
    return np.ascontiguousarray(a).astype(ml_dtypes.bfloat16)


def _wrap16(a):
    w = a.reshape(-1, 16).T
    return np.ascontiguousarray(np.tile(w, (8, 1)))


def _cell_streams(src, dst, dst_base, rng_len):
    """Split one core's edges into (src-bucket, dst-subbucket) cells; each cell
    is dst-sorted with pads (-1) so no equal-dst group crosses a 128 boundary
    and the cell is a multiple of 128. Returns {(sb, db): int64 stream}."""
    sb = src // SPAN
    dl = dst - dst_base
    db = dl // SPAN
    cells = {}
    order = np.lexsort((src, dl, db, sb))
    for key in sorted(set(zip(sb[order].tolist(), db[order].tolist()))):
        idx = order[(sb[order] == key[0]) & (db[order] == key[1])]
        d = dl[idx]
        out = []
        k = 0
        while k < len(idx):
            g = k
            while g < len(idx) and d[g] == d[k]:
                g += 1
            glen = g - k
            assert glen <= P, f"dst group of size {glen} exceeds 128"
            if (len(out) % P) + glen > P:
                out.extend([-1] * (P - len(out) % P))
            out.extend(idx[k:g].tolist())
            k = g
        if len(out) % P:
            out.extend([-1] * (P - len(out) % P))
        cells[key] = np.array(out, np.int64)
    return cells


def _grid(all_cells):
    """Union of cell keys with per-cell max tile count across cores."""
    keys = sorted(set().union(*[c.keys() for c in all_cells]))
    sizes = {k: max(len(c.get(k, ())) // P for c in all_cells) for k in keys}
    return keys, sizes


def _phase_prep(per_core_src_dst, dst_base_of, rng_len, featT_of):
    """Build the shared stream layout + per-core arrays for one cgcnn phase.

    per_core_src_dst: list of (src_rows, dst_pos, local_edge_sel) per core.
    featT_of(k): [64, n_all_edges]-like bf16 source for per-edge features.
    Returns (meta, per_core dict arrays).
    """
    all_cells = []
    for k in range(NC):
        src, dst, _ = per_core_src_dst[k]
        all_cells.append(_cell_streams(src, dst, dst_base_of(k), rng_len))
    keys, sizes = _grid(all_cells)
    L = sum(sizes[k] for k in keys) * P
    n_dsub = int(np.ceil(rng_len / SPAN))
    # rows per dst-subbucket table (+1 trash each)
    dsub_rows = [min(SPAN, rng_len - s * SPAN) + 1 for s in range(n_dsub)]
    dsub_base = np.concatenate([[0], np.cumsum(dsub_rows)]).astype(int)

    # call list: chunks of tiles within each cell
    calls = []
    t0 = 0
    for key in keys:
        nt_cell = sizes[key]
        t = 0
        while t < nt_cell:
            nt = min(CALL_TILES, nt_cell - t)
            calls.append(dict(t0=t0 + t, nt=nt, sbase=key[0] * SPAN,
                              dbase=key[1] * SPAN, dsub=key[1]))
            t += nt
        t0 += nt_cell

    meta = dict(L=L, calls=calls, n_dsub=n_dsub, dsub_rows=dsub_rows,
                dsub_base=dsub_base.tolist(), keys=keys, sizes=sizes)

    cores = []
    for k in range(NC):
        src, dst, sel = per_core_src_dst[k]
        dl = dst - dst_base_of(k)
        stream = np.full(L, -1, np.int64)
        t0 = 0
        for key in keys:
            cell = all_cells[k].get(key)
            if cell is not None:
                stream[t0 * P: t0 * P + len(cell)] = cell
            t0 += sizes[key]
        real = stream >= 0
        e = np.where(real, stream, 0)
        s_src = np.where(real, src[e], 0)
        s_dl = np.where(real, dl[e], 0)

        gsrc = np.zeros(L, np.int16)
        gdst = np.zeros(L, np.int16)
        sidx = np.zeros(L, np.int16)
        for c in calls:
            a, b = c["t0"] * P, (c["t0"] + c["nt"]) * P
            r = real[a:b]
            gsrc[a:b] = np.where(r, s_src[a:b] - c["sbase"], 0).astype(np.int16)
            gdst[a:b] = np.where(r, s_dl[a:b] - c["dbase"], 0).astype(np.int16)
            trash = dsub_rows[c["dsub"]] - 1
            rel = np.where(r, s_dl[a:b] - c["dbase"], -7)
            first = np.ones(b - a, bool)
            first[1:] = rel[1:] != rel[:-1]
            sidx[a:b] = np.where(r & first, rel, trash).astype(np.int16)

        fT = featT_of(k)
        zT = np.zeros((NF, L), ml_dtypes.bfloat16)
        zT[:, real] = fT[:, e[real]]
        dstrel = np.where(real, s_dl.astype(np.float32), -1.0)

        cores.append(dict(
            gsrc=_wrap16(gsrc), gdst=_wrap16(gdst), sidx=_wrap16(sidx),
            zT=zT, dstrel=dstrel.reshape(1, L).astype(np.float32),
            stream=stream, real=real, sel=sel,
        ))
    return meta, cores


def prep(x, y, z, params, g_src, g_dst, lg_src, lg_dst, cfg=None):
    cfg = cfg or CFG
    N, E, LG = cfg["n_nodes"], cfg["n_edges"], cfg["n_lg_edges"]
    npc = N // NC
    assert npc * NC == N

    x = np.asarray(x, np.float32)
    y = np.asarray(y, np.float32)
    z = np.asarray(z, np.float32)
    yT = _bf16(y.T)
    zT = _bf16(z.T)

    # ---------------- phase 1 (node update on g)
    owner1 = np.asarray(g_dst) // npc
    pcs = []
    for k in range(NC):
        sel = np.where(owner1 == k)[0]
        pcs.append((np.asarray(g_src)[sel], np.asarray(g_dst)[sel], sel))
    meta1, cores1 = _phase_prep(
        pcs, lambda k: k * npc, npc,
        lambda k: yT[:, pcs[k][2]])
    L1 = meta1["L"]

    # slot map: g-edge id -> global padded slot (core*L1 + pos)
    slot = np.full(E, -1, np.int64)
    for k in range(NC):
        st = cores1[k]["stream"]
        r = cores1[k]["real"]
        slot[pcs[k][2][st[r]]] = k * L1 + np.where(r)[0]
    assert (slot >= 0).all()

    # per-core y rows in stream order (for P2 residual)
    for k in range(NC):
        st, r = cores1[k]["stream"], cores1[k]["real"]
        rows = np.zeros((L1, NF), np.float32)
        rows[r] = y[pcs[k][2][st[r]]]
        cores1[k]["y_own"] = rows

    # ---------------- phase 3 (edge update on lg, over padded slots)
    src_slot = slot[np.asarray(lg_src)]
    dst_slot = slot[np.asarray(lg_dst)]
    owner3 = dst_slot // L1
    pcs3 = []
    for k in range(NC):
        sel = np.where(owner3 == k)[0]
        pcs3.append((src_slot[sel], dst_slot[sel], sel))
    meta3, cores3 = _phase_prep(
        pcs3, lambda k: k * L1, L1,
        lambda k: zT[:, pcs3[k][2]])

    meta = dict(cfg=cfg, npc=npc, L1=L1, meta1=meta1, meta3=meta3,
                E=E, LG=LG, N=N)

    # ---------------- params (biases cancel inside BN; only weights + bn)
    pn, pe = params["node"], params["edge"]
    wsd1 = _bf16(np.concatenate([np.asarray(pn["W_src"]), np.asarray(pn["W_dst"])], 0))
    we1 = _bf16(np.asarray(pn["W_edge"]))
    wsd2 = _bf16(np.concatenate([np.asarray(pe["W_src"]), np.asarray(pe["W_dst"])], 0))
    we2 = _bf16(np.asarray(pe["W_edge"]))
    wb = _bf16(np.asarray(params["Wb"]))
    v = lambda a: np.asarray(a, np.float32).reshape(1, -1)

    shared = dict(
        x_full=x, wsd1=wsd1, we1=we1, wsd2=wsd2, we2=we2, wb=wb,
        bnm1_g=v(pn["bn_m_g"]), bnm1_b=v(pn["bn_m_b"]),
        bn1_g=v(pn["bn_g"]), bn1_b=v(pn["bn_b"]),
        bnm2_g=v(pe["bn_m_g"]), bnm2_b=v(pe["bn_m_b"]),
        bn2_g=v(pe["bn_g"]), bn2_b=v(pe["bn_b"]),
        bnb_g=v(params["bn_b_g"]), bnb_b=v(params["bn_b_b"]),
    )
    in_maps = []
    for k in range(NC):
        m = dict(shared)
        m["x_own"] = x[k * npc:(k + 1) * npc]
        c1, c3 = cores1[k], cores3[k]
        m.update(g1_src=c1["gsrc"], g1_dst=c1["gdst"], g1_sidx=c1["sidx"],
                 yT_s=c1["zT"], drel1=c1["dstrel"], y_own=c1["y_own"],
                 g3_src=c3["gsrc"], g3_dst=c3["gdst"], g3_sidx=c3["sidx"],
                 zT_s=c3["zT"], drel3=c3["dstrel"])
        in_maps.append(m)

    post = dict(cores1=cores1, cores3=cores3, pcs=pcs, pcs3=pcs3, slot=slot)
    return meta, in_maps, post


# =============================================================== device build

def build(meta, sim_mode=False):
    cfg = meta["cfg"]
    N, E, LG = cfg["n_nodes"], cfg["n_edges"], cfg["n_lg_edges"]
    npc, L1 = meta["npc"], meta["L1"]
    m1, m3 = meta["meta1"], meta["meta3"]
    L3 = m3["L"]
    H1 = int(np.ceil(m1["dsub_base"][-1] / 1024)) * 1024
    H3 = int(np.ceil(m3["dsub_base"][-1] / 1024)) * 1024

    nc = bacc.Bacc("TRN2", target_bir_lowering=False, debug=False,
                   num_devices=1 if sim_mode else NC)

    din = lambda n, s, d: nc.dram_tensor(n, s, d, kind="ExternalInput")
    x_full = din("x_full", [N, NF], F32)
    x_own = din("x_own", [npc, NF], F32)
    wsd1 = din("wsd1", [P, 2 * NF], BF16)
    we1 = din("we1", [NF, 2 * NF], BF16)
    wsd2 = din("wsd2", [P, 2 * NF], BF16)
    we2 = din("we2", [NF, 2 * NF], BF16)
    wb = din("wb", [P, NF], BF16)
    bn = {n: din(n, [1, 2 * NF if "m" in n else NF], F32)
          for n in ["bnm1_g", "bnm1_b", "bn1_g", "bn1_b", "bnm2_g", "bnm2_b",
                    "bn2_g", "bn2_b", "bnb_g", "bnb_b"]}
    g1_src = din("g1_src", [P, L1 // 16], I16)
    g1_dst = din("g1_dst", [P, L1 // 16], I16)
    g1_sidx = din("g1_sidx", [P, L1 // 16], I16)
    yT_s = din("yT_s", [NF, L1], BF16)
    drel1 = din("drel1", [1, L1], F32)
    y_own = din("y_own", [L1, NF], F32)
    g3_src = din("g3_src", [P, L3 // 16], I16)
    g3_dst = din("g3_dst", [P, L3 // 16], I16)
    g3_sidx = din("g3_sidx", [P, L3 // 16], I16)
    zT_s = din("zT_s", [NF, L3], BF16)
    drel3 = din("drel3", [1, L3], F32)

    xout = nc.dram_tensor("xout", [npc, NF], F32, kind="ExternalOutput")
    yout = nc.dram_tensor("yout", [L1, NF], F32, kind="ExternalOutput")

    mpre1 = nc.dram_tensor("mpre1", [P, L1], BF16, kind="Internal")
    mpre3 = nc.dram_tensor("mpre3", [P, L3], BF16, kind="Internal")
    mT = nc.dram_tensor("mT", [NF, L1], BF16, kind="Internal")
    t2d = nc.dram_tensor("t2d", [L1, NF], BF16, kind="Internal")
    y2m = nc.dram_tensor("y2m", [L1, NF], F32, kind="Internal")
    y2tab = nc.dram_tensor("y2tab", [NC * L1, NF], F32, kind="Internal",
                           addr_space="Shared")
    htab1 = nc.dram_tensor("htab1", [H1, NF], F32, kind="Internal")
    htab3 = nc.dram_tensor("htab3", [H3, NF], F32, kind="Internal")
    arin = nc.dram_tensor("arin", [1, 512], F32, kind="Internal")
    arout = nc.dram_tensor("arout", [1, 512], F32, kind="Internal",
                           addr_space="Shared")

    from contextlib import ExitStack
    with tile.TileContext(nc) as tc, ExitStack() as stack:
        sbc = stack.enter_context(tc.tile_pool(name="sbc", bufs=1))   # constants
        sb = stack.enter_context(tc.tile_pool(name="sb", bufs=2))
        pstat = stack.enter_context(tc.tile_pool(name="pstat", bufs=1, space="PSUM"))
        ps = stack.enter_context(tc.tile_pool(name="ps", bufs=2, space="PSUM"))
        ps3 = stack.enter_context(tc.tile_pool(name="ps3", bufs=2, space="PSUM"))

        ident = sbc.tile([P, P], F32)
        make_identity(nc, ident[:])
        ones1 = sbc.tile([1, P], F32)
        nc.vector.memset(ones1[:], 1.0)
        onescol = sbc.tile([1, 1], F32)
        nc.vector.memset(onescol[:], 1.0)
        ones128 = sbc.tile([P, 1], F32)
        nc.vector.memset(ones128[:], 1.0)

        def load_const(t, dt=BF16, shape=None):
            h = sbc.tile(shape or list(t.shape), dt, tag=t.name)
            nc.sync.dma_start(h[:], t[:])
            return h

        wsd1_s = load_const(wsd1); we1_s = load_const(we1)
        wsd2_s = load_const(wsd2); we2_s = load_const(we2)
        wb_s = load_const(wb)
        bn_s = {n: load_const(t, F32) for n, t in bn.items()}

        # ---- zero-init scatter tables
        zt = sbc.tile([P, 8 * NF], F32)
        nc.vector.memset(zt[:], 0.0)
        for tab, H in ((htab1, H1), (htab3, H3)):
            r = 0
            while r < H:
                n = min(1024, H - r)
                nc.sync.dma_start(tab[r:r + n, :].opt(), zt[:, :n * NF // P].opt())
                r += n

        # ---- helpers
        def bcast_row(vec_ap, n):
            """[1, n] sbuf -> [128, n] f32 sbuf via ones-matmul."""
            pb = ps3.tile([P, n], F32, space="PSUM", tag="aux")
            nc.tensor.matmul(out=pb[:], lhsT=ones1[:], rhs=vec_ap, start=True, stop=True)
            out = sbc.tile([P, n], F32, tag=f"bc{len(tc.nc.m.functions[0].allocations)}")
            nc.vector.tensor_copy(out[:], pb[:])
            return out

        def stats_finalize(acc_s, acc_q, n_feat, divisor, g_ap, b_ap, slot_off):
            """acc_s/acc_q: [128, n_feat] partial sums over partitions*  (already
            summed over tiles into 128-partition accumulators). Reduce over
            partitions via ones-matmul, then pack into arin at slot_off."""
            red = pstat.tile([1, 2 * n_feat], F32, space="PSUM", tag="red")
            nc.tensor.matmul(out=red[:, :n_feat], lhsT=ones128[:], rhs=acc_s[:],
                             start=True, stop=True)
            nc.tensor.matmul(out=red[:, n_feat:], lhsT=ones128[:], rhs=acc_q[:],
                             start=True, stop=True)
            pk = sb.tile([1, 2 * n_feat], F32, tag="pk")
            nc.vector.tensor_copy(pk[:], red[:])
            nc.sync.dma_start(arin[:, slot_off:slot_off + 2 * n_feat], pk[:])

        def allreduce_scaleshift(n_feat, divisor, g_h, b_h, slot_off):
            """After arin is filled: AllReduce, then compute per-feature
            scale/shift rows [1, n_feat] and broadcast to [128, n_feat]."""
            if sim_mode:
                nc.sync.dma_start(arout[:], arin[:])
            else:
                nc.gpsimd.collective_compute(
                    "AllReduce", OP.add, replica_groups=[list(range(NC))],
                    ins=[arin[:].opt()], outs=[arout[:].opt()])
            st = sb.tile([1, 2 * n_feat], F32, tag="st")
            nc.sync.dma_start(st[:], arout[:, slot_off:slot_off + 2 * n_feat])
            mean = sb.tile([1, n_feat], F32, tag="mean")
            nc.vector.tensor_scalar(out=mean[:], in0=st[:, :n_feat],
                                    scalar1=1.0 / divisor, scalar2=None, op0=OP.mult)
            ex2 = sb.tile([1, n_feat], F32, tag="ex2")
            nc.vector.tensor_scalar(out=ex2[:], in0=st[:, n_feat:],
                                    scalar1=1.0 / divisor, scalar2=None, op0=OP.mult)
            var = sb.tile([1, n_feat], F32, tag="var")
            nc.vector.tensor_tensor(out=var[:], in0=mean[:], in1=mean[:], op=OP.mult)
            nc.vector.tensor_tensor(out=var[:], in0=ex2[:], in1=var[:],
                                    op=OP.subtract)
            lnv = sb.tile([1, n_feat], F32, tag="lnv")
            nc.vector.tensor_scalar(out=var[:], in0=var[:], scalar1=EPS,
                                    scalar2=None, op0=OP.add)
            nc.scalar.activation(lnv[:], var[:], AF.Ln)
            istd = sb.tile([1, n_feat], F32, tag="istd")
            nc.scalar.activation(istd[:], lnv[:], AF.Exp, scale=-0.5)
            scale = sb.tile([1, n_feat], F32, tag="scl")
            nc.vector.tensor_tensor(out=scale[:], in0=g_h[:], in1=istd[:], op=OP.mult)
            shift = sb.tile([1, n_feat], F32, tag="shf")
            nc.vector.tensor_tensor(out=shift[:], in0=mean[:], in1=scale[:], op=OP.mult)
            nc.vector.tensor_tensor(out=shift[:], in0=b_h[:], in1=shift[:],
                                    op=OP.subtract)
            return (bcast_row(scale[:], n_feat), bcast_row(shift[:], n_feat),
                    scale, shift)

        def gate_tiles(src_ap, nt, t0, scale_b, shift_b, drel_h, out_cb):
            """Apply BN + sigmoid*softplus gate to m tiles [128,128]bf16 read
            from src_ap (sbuf [128, nt*128]); returns gated [128, nt*64] f32 in
            out_cb; also per-tile (dstrelT, mask) for sel/scatter use."""
            pass

        # ================= generic cgcnn edge pipeline =================
        def edge_phase(pm, gsrc_t, gdst_t, sidx_t, featT_t, drel_t, mpre_t,
                       src_tab, dst_tab, wsd_h, we_h, bnm_g, bnm_b,
                       htab, n_edges_real, store_mT):
            """Pass 1: project+stats; returns scale/shift bcasts after AR.
            Then pass 2: gate + scatter (+ mT store)."""
            L = pm["L"]
            acc_s = sbc.tile([P, 2 * NF], F32, tag=f"acs{store_mT}")
            acc_q = sbc.tile([P, 2 * NF], F32, tag=f"acq{store_mT}")
            nc.vector.memset(acc_s[:], 0.0)
            nc.vector.memset(acc_q[:], 0.0)

            for c in pm["calls"]:
                t0, nt, sbase, dbase = c["t0"], c["nt"], c["sbase"], c["dbase"]
                e0, n_e = t0 * P, nt * P
                span_s = min(SPAN, src_tab.shape[0] - sbase)
                span_d = min(SPAN, dst_tab.shape[0] - dbase)
                si1 = sb.tile([P, CALL_TILES * 8], I16, tag="si1")
                nc.sync.dma_start(si1[:, :n_e // 16], gsrc_t[:, e0 // 16:(e0 + n_e) // 16])
                si2 = sb.tile([P, CALL_TILES * 8], I16, tag="si2")
                nc.sync.dma_start(si2[:, :n_e // 16], gdst_t[:, e0 // 16:(e0 + n_e) // 16])
                xs = sb.tile([P, CALL_TILES, NF], F32, tag="xs")
                nc.gpsimd.dma_gather(
                    out_ap=xs[:, :nt, :], in_ap=src_tab[sbase:sbase + span_s, :],
                    idxs_ap=si1[:, :n_e // 16],
                    num_idxs=n_e, num_idxs_reg=n_e, elem_size=NF,
                    single_packet=False)
                xd = sb.tile([P, CALL_TILES, NF], F32, tag="xd")
                nc.gpsimd.dma_gather(
                    out_ap=xd[:, :nt, :], in_ap=dst_tab[dbase:dbase + span_d, :],
                    idxs_ap=si2[:, :n_e // 16],
                    num_idxs=n_e, num_idxs_reg=n_e, elem_size=NF,
                    single_packet=False)
                zt_c = sb.tile([NF, CALL_TILES * P], BF16, tag="ztc")
                nc.sync.dma_start(zt_c[:, :n_e], featT_t[:, e0:e0 + n_e])
                dr_c = sb.tile([1, CALL_TILES * P], F32, tag="drc")
                nc.sync.dma_start(dr_c[:, :n_e], drel_t[:, e0:e0 + n_e])
                mch = sb.tile([P, CALL_TILES * P], BF16, tag="mch")
                for t in range(nt):
                    es = t * P
                    tp1 = ps.tile([NF, P], F32, space="PSUM", tag="tp")
                    nc.tensor.transpose(out=tp1[:], in_=xs[:, t, :], identity=ident[:])
                    tp2 = ps.tile([NF, P], F32, space="PSUM", tag="tp")
                    nc.tensor.transpose(out=tp2[:], in_=xd[:, t, :], identity=ident[:])
                    combo = sb.tile([P, P], BF16, tag="combo")
                    nc.vector.tensor_copy(combo[:NF, :], tp1[:])
                    nc.vector.tensor_copy(combo[NF:, :], tp2[:])
                    mm = ps.tile([P, P], F32, space="PSUM", tag="mm")
                    nc.tensor.matmul(out=mm[:], lhsT=combo[:], rhs=wsd_h[:],
                                     start=True, stop=False)
                    nc.tensor.matmul(out=mm[:], lhsT=zt_c[:, es:es + P],
                                     rhs=we_h[:], start=False, stop=True)
                    # mask column: dstrel >= 0 (pads excluded from stats via lhsT)
                    drT = ps3.tile([P, 1], F32, space="PSUM", tag="aux")
                    nc.tensor.matmul(out=drT[:], lhsT=dr_c[:, es:es + P],
                                     rhs=onescol[:], start=True, stop=True)
                    mk = sb.tile([P, 1], BF16, tag="mk")
                    nc.vector.tensor_scalar(out=mk[:], in0=drT[:], scalar1=0.0,
                                            scalar2=None, op0=OP.is_ge)
                    nc.vector.tensor_copy(mch[:, es:es + P], mm[:])
                    sq = sb.tile([P, P], BF16, tag="sq")
                    nc.scalar.activation(sq[:], mm[:], AF.Square)
                    nc.vector.scalar_tensor_tensor(
                        out=acc_s[:], in0=mch[:, es:es + P], scalar=mk[:],
                        in1=acc_s[:], op0=OP.mult, op1=OP.add)
                    nc.vector.scalar_tensor_tensor(
                        out=acc_q[:], in0=sq[:], scalar=mk[:],
                        in1=acc_q[:], op0=OP.mult, op1=OP.add)
                nc.sync.dma_start(mpre_t[:, e0:e0 + n_e], mch[:, :n_e])

            sred = pstat.tile([1, 4 * NF], F32, space="PSUM", tag="sred")
            nc.tensor.matmul(out=sred[:, :2 * NF], lhsT=ones128[:], rhs=acc_s[:],
                             start=True, stop=True)
            nc.tensor.matmul(out=sred[:, 2 * NF:], lhsT=ones128[:], rhs=acc_q[:],
                             start=True, stop=True)
            pk = sb.tile([1, 4 * NF], F32, tag="pk")
            nc.vector.tensor_copy(pk[:], sred[:])
            nc.sync.dma_start(arin[:, 0:4 * NF], pk[:])
            scale_b, shift_b, _, _ = allreduce_scaleshift(2 * NF, n_edges_real,
                                                    bn_s[bnm_g], bn_s[bnm_b], 0)

            # ---------------- pass 2
            for c in pm["calls"]:
                t0, nt, dbase, dsub = c["t0"], c["nt"], c["dbase"], c["dsub"]
                e0, n_e = t0 * P, nt * P
                hbase = int(pm["dsub_base"][dsub])
                hrows = pm["dsub_rows"][dsub]
                mch = sb.tile([P, CALL_TILES * P], BF16, tag="mch2")
                nc.sync.dma_start(mch[:, :n_e], mpre_t[:, e0:e0 + n_e])
                if store_mT:
                    mTch = sb.tile([NF, CALL_TILES * P], BF16, tag="mTch")
                dr_c = sb.tile([1, CALL_TILES * P], F32, tag="drc2")
                nc.sync.dma_start(dr_c[:, :n_e], drel_t[:, e0:e0 + n_e])
                comb = sb.tile([P, CALL_TILES, NF], F32, tag="comb")
                for t in range(nt):
                    es = t * P
                    t1 = sb.tile([P, P], F32, tag="t1")
                    nc.vector.tensor_tensor(out=t1[:], in0=mch[:, es:es + P],
                                            in1=scale_b[:], op=OP.mult)
                    nc.vector.tensor_tensor(out=t1[:], in0=t1[:], in1=shift_b[:],
                                            op=OP.add)
                    e1 = sb.tile([P, NF], F32, tag="e1")
                    nc.scalar.activation(e1[:], t1[:, :NF], AF.Exp, scale=-1.0)
                    nc.vector.tensor_scalar(out=e1[:], in0=e1[:], scalar1=1.0,
                                            scalar2=None, op0=OP.add)
                    sg = sb.tile([P, NF], F32, tag="sg")
                    nc.vector.reciprocal(sg[:], e1[:])
                    e2 = sb.tile([P, NF], F32, tag="e2")
                    nc.scalar.activation(e2[:], t1[:, NF:], AF.Exp)
                    sp = sb.tile([P, NF], F32, tag="sp")
                    nc.scalar.activation(sp[:], e2[:], AF.Ln, bias=1.0)
                    drT = ps3.tile([P, 1], F32, space="PSUM", tag="aux")
                    nc.tensor.matmul(out=drT[:], lhsT=dr_c[:, es:es + P],
                                     rhs=onescol[:], start=True, stop=True)
                    drTs = sb.tile([P, 1], F32, tag="drTs")
                    nc.vector.tensor_copy(drTs[:], drT[:])
                    mk = sb.tile([P, 1], F32, tag="mk2")
                    nc.vector.tensor_scalar(out=mk[:], in0=drTs[:], scalar1=0.0,
                                            scalar2=None, op0=OP.is_ge)
                    g = sb.tile([P, NF], F32, tag="g")
                    nc.vector.scalar_tensor_tensor(
                        out=g[:], in0=sg[:], scalar=mk[:], in1=sp[:],
                        op0=OP.mult, op1=OP.mult)
                    if store_mT:
                        gT = ps3.tile([NF, P], F32, space="PSUM", tag="aux")
                        nc.tensor.transpose(out=gT[:], in_=g[:], identity=ident[:])
                        nc.vector.tensor_copy(mTch[:, es:es + P], gT[:])
                    # selection matrix: sel[i,j] = (dst_i == dst_j)
                    rb = ps3.tile([P, P], F32, space="PSUM", tag="aux")
                    nc.tensor.matmul(out=rb[:], lhsT=ones1[:],
                                     rhs=dr_c[:, es:es + P], start=True, stop=True)
                    sel = sb.tile([P, P], F32, tag="sel")
                    nc.vector.tensor_scalar(out=sel[:], in0=rb[:], scalar1=drTs[:],
                                            scalar2=None, op0=OP.is_equal)
                    cp = ps3.tile([P, NF], F32, space="PSUM", tag="aux")
                    nc.tensor.matmul(out=cp[:], lhsT=sel[:], rhs=g[:],
                                     start=True, stop=True)
                    nc.vector.tensor_copy(comb[:, t, :], cp[:])
                si3 = sb.tile([P, CALL_TILES * 8], I16, tag="si3")
                nc.sync.dma_start(si3[:, :n_e // 16], sidx_t[:, e0 // 16:(e0 + n_e) // 16])
                if store_mT:
                    nc.sync.dma_start(mT[:, e0:e0 + n_e], mTch[:, :n_e])
                nc.gpsimd.dma_scatter_add(
                    out_ap=htab[hbase:hbase + hrows, :], in_ap=comb[:, :nt, :],
                    idxs_ap=si3[:, :n_e // 16],
                    num_idxs=n_e, num_idxs_reg=n_e, elem_size=NF,
                    single_packet=False)

        # =================== node epilogue (h BN + softplus residual)
        def node_epilogue(htab, pm, rng_len, n_real, gname, bname, slot_off,
                          res_rows, out_t, extra_res=None):
            """stats over htab real rows; out = softplus(res + BN(h))."""
            ECH = 8
            acc_s = sbc.tile([P, ECH * NF], F32, tag=f"eas{slot_off}")
            acc_q = sbc.tile([P, ECH * NF], F32, tag=f"eaq{slot_off}")
            nc.vector.memset(acc_s[:], 0.0)
            nc.vector.memset(acc_q[:], 0.0)
            slabs = []   # (htab_row, pos, ch)  ch full 128-tiles per slab
            tails = []   # (htab_row, pos, n)
            for s in range(pm["n_dsub"] if pm else 1):
                base = int(pm["dsub_base"][s]) if pm else 0
                rows = (pm["dsub_rows"][s] - 1) if pm else rng_len
                r = 0
                while r + P <= rows:
                    ch = min(ECH, (rows - r) // P)
                    slabs.append((base + r, s * SPAN + r, ch))
                    r += ch * P
                if r < rows:
                    tails.append((base + r, s * SPAN + r, rows - r))

            def slab_ap(tensor_ap, row0, ch):
                return bass.AP(tensor_ap.ap().tensor, row0 * NF,
                               [[NF, P], [P * NF, ch], [1, NF]])

            for (hr, pos, ch) in slabs:
                ht = sb.tile([P, ECH, NF], F32, tag="htS")
                nc.sync.dma_start(ht[:, :ch, :], slab_ap(htab, hr, ch))
                nc.vector.tensor_tensor(out=acc_s[:, :ch * NF], in0=acc_s[:, :ch * NF],
                                        in1=ht[:, :ch, :], op=OP.add)
                sq = sb.tile([P, ECH, NF], F32, tag="esqS")
                nc.scalar.activation(sq[:, :ch, :], ht[:, :ch, :], AF.Square)
                nc.vector.tensor_tensor(out=acc_q[:, :ch * NF], in0=acc_q[:, :ch * NF],
                                        in1=sq[:, :ch, :], op=OP.add)
            for (hr, pos, n) in tails:
                ht = sb.tile([P, NF], F32, tag="ht")
                nc.sync.dma_start(ht[:n, :], htab[hr:hr + n, :])
                nc.vector.tensor_tensor(out=acc_s[:n, :NF], in0=acc_s[:n, :NF],
                                        in1=ht[:n, :], op=OP.add)
                sq = sb.tile([P, NF], F32, tag="esq")
                nc.scalar.activation(sq[:n, :], ht[:n, :], AF.Square)
                nc.vector.tensor_tensor(out=acc_q[:n, :NF], in0=acc_q[:n, :NF],
                                        in1=sq[:n, :], op=OP.add)
            # fold ECH blocks into one [P, NF] then reduce partitions
            for b in range(1, ECH):
                nc.vector.tensor_tensor(out=acc_s[:, :NF], in0=acc_s[:, :NF],
                                        in1=acc_s[:, b * NF:(b + 1) * NF], op=OP.add)
                nc.vector.tensor_tensor(out=acc_q[:, :NF], in0=acc_q[:, :NF],
                                        in1=acc_q[:, b * NF:(b + 1) * NF], op=OP.add)
            stats_finalize(acc_s[:, :NF], acc_q[:, :NF], NF, n_real, gname,
                           bname, slot_off)
            scale_b, shift_b, _, _ = allreduce_scaleshift(NF, n_real, bn_s[gname],
                                                    bn_s[bname], slot_off)
            sc3 = bass.AP(scale_b[:].tensor, scale_b[:].offset,
                          [scale_b[:].ap[0], [0, ECH], scale_b[:].ap[1]])
            sh3 = bass.AP(shift_b[:].tensor, shift_b[:].offset,
                          [shift_b[:].ap[0], [0, ECH], shift_b[:].ap[1]])
            for (hr, pos, ch) in slabs:
                ht = sb.tile([P, ECH, NF], F32, tag="ht2S")
                nc.sync.dma_start(ht[:, :ch, :], slab_ap(htab, hr, ch))
                sc3c = bass.AP(sc3.tensor, sc3.offset,
                               [sc3.ap[0], [0, ch], sc3.ap[2]])
                sh3c = bass.AP(sh3.tensor, sh3.offset,
                               [sh3.ap[0], [0, ch], sh3.ap[2]])
                nc.vector.tensor_tensor(out=ht[:, :ch, :], in0=ht[:, :ch, :],
                                        in1=sc3c, op=OP.mult)
                nc.vector.tensor_tensor(out=ht[:, :ch, :], in0=ht[:, :ch, :],
                                        in1=sh3c, op=OP.add)
                rr = sb.tile([P, ECH, NF], F32, tag="rrS")
                nc.sync.dma_start(rr[:, :ch, :], slab_ap(res_rows, pos, ch))
                nc.vector.tensor_tensor(out=ht[:, :ch, :], in0=ht[:, :ch, :],
                                        in1=rr[:, :ch, :], op=OP.add)
                ex = sb.tile([P, ECH, NF], F32, tag="exS")
                nc.scalar.activation(ex[:, :ch, :], ht[:, :ch, :], AF.Exp)
                nc.scalar.activation(ht[:, :ch, :], ex[:, :ch, :], AF.Ln, bias=1.0)
                nc.sync.dma_start(slab_ap(out_t, pos, ch), ht[:, :ch, :])
            for (hr, pos, n) in tails:
                ht = sb.tile([P, NF], F32, tag="ht2")
                nc.sync.dma_start(ht[:n, :], htab[hr:hr + n, :])
                nc.vector.tensor_tensor(out=ht[:n, :], in0=ht[:n, :],
                                        in1=scale_b[:n, :], op=OP.mult)
                nc.vector.tensor_tensor(out=ht[:n, :], in0=ht[:n, :],
                                        in1=shift_b[:n, :], op=OP.add)
                rr = sb.tile([P, NF], F32, tag="rr")
                nc.sync.dma_start(rr[:n, :], res_rows[pos:pos + n, :])
                nc.vector.tensor_tensor(out=ht[:n, :], in0=ht[:n, :],
                                        in1=rr[:n, :], op=OP.add)
                ex = sb.tile([P, NF], F32, tag="ex")
                nc.scalar.activation(ex[:n, :], ht[:n, :], AF.Exp)
                nc.scalar.activation(ht[:n, :], ex[:n, :], AF.Ln, bias=1.0)
                nc.sync.dma_start(out_t[pos:pos + n, :], ht[:n, :])

        # ========================== run phases ==========================
        edge_phase(m1, g1_src, g1_dst, g1_sidx, yT_s, drel1, mpre1,
                   x_full, x_own, wsd1_s, we1_s, "bnm1_g", "bnm1_b",
                   htab1, E, store_mT=True)
        node_epilogue(htab1, None, npc, N, "bn1_g", "bn1_b", 256,
                      x_own, xout)

        # ---------------- phase 2: edge bottleneck (transposed-t layout)
        CH = 8
        ntile1 = L1 // P
        t2dT = nc.dram_tensor("t2dT", [NF, L1], BF16, kind="Internal")
        sred2 = pstat.tile([1, 2 * NF], F32, space="PSUM", tag="sred")
        acc2 = sbc.tile([P, NF], F32, tag="p2acc")
        nc.vector.memset(acc2[:], 0.0)
        acq2 = sbc.tile([P, NF], F32, tag="p2acq")
        nc.vector.memset(acq2[:], 0.0)
        tglob = 0
        for s0 in range(0, ntile1, CH):
            ch = min(CH, ntile1 - s0)
            e0 = s0 * P
            ym_slab = sb.tile([P, CH * P], BF16, tag="ymslab")
            nc.sync.dma_start(ym_slab[:NF, :ch * P], yT_s[:, e0:e0 + ch * P])
            nc.sync.dma_start(ym_slab[NF:, :ch * P], mT[:, e0:e0 + ch * P])
            ttT_slab = sb.tile([NF, CH * P], BF16, tag="ttTslab")
            for t in range(ch):
                es = t * P
                tt = ps.tile([P, NF], F32, space="PSUM", tag="mm")
                nc.tensor.matmul(out=tt[:], lhsT=ym_slab[:, es:es + P],
                                 rhs=wb_s[:], start=True, stop=True)
                tts = sb.tile([P, NF], F32, tag="tts")
                nc.vector.tensor_copy(tts[:], tt[:])
                sq = sb.tile([P, NF], F32, tag="p2sq")
                nc.scalar.activation(sq[:], tt[:], AF.Square)
                nc.vector.tensor_tensor(out=acc2[:], in0=acc2[:], in1=tts[:],
                                        op=OP.add)
                nc.vector.tensor_tensor(out=acq2[:], in0=acq2[:], in1=sq[:],
                                        op=OP.add)
                tpT = ps.tile([NF, P], F32, space="PSUM", tag="tp")
                nc.tensor.transpose(out=tpT[:], in_=tts[:], identity=ident[:])
                nc.vector.tensor_copy(ttT_slab[:, es:es + P], tpT[:])
                tglob += 1
            nc.sync.dma_start(t2dT[:, e0:e0 + ch * P], ttT_slab[:, :ch * P])
        nc.tensor.matmul(out=sred2[:, :NF], lhsT=ones128[:], rhs=acc2[:],
                         start=True, stop=True)
        nc.tensor.matmul(out=sred2[:, NF:], lhsT=ones128[:], rhs=acq2[:],
                         start=True, stop=True)
        pk2 = sb.tile([1, 2 * NF], F32, tag="pk")
        nc.vector.tensor_copy(pk2[:], sred2[:])
        nc.sync.dma_start(arin[:, 384:384 + 2 * NF], pk2[:])
        _, _, p2_scale, p2_shift = allreduce_scaleshift(
            NF, E, bn_s["bnb_g"], bn_s["bnb_b"], 384)
        # column versions [NF, 1] via DRAM round-trip transpose
        ssd = nc.dram_tensor("ssd", [2, NF], F32, kind="Internal")
        nc.sync.dma_start(ssd[0:1, :], p2_scale[:])
        nc.sync.dma_start(ssd[1:2, :], p2_shift[:])
        scale_col = sbc.tile([NF, 1], F32, tag="sccol")
        nc.sync.dma_start(scale_col[:], bass.AP(ssd.ap().tensor, 0, [[1, NF], [1, 1]]))
        shift_col = sbc.tile([NF, 1], F32, tag="shcol")
        nc.sync.dma_start(shift_col[:], bass.AP(ssd.ap().tensor, NF, [[1, NF], [1, 1]]))
        for s0 in range(0, ntile1, CH):
            ch = min(CH, ntile1 - s0)
            e0 = s0 * P
            ttT_slab = sb.tile([NF, CH * P], BF16, tag="ttTslab2")
            nc.sync.dma_start(ttT_slab[:, :ch * P], t2dT[:, e0:e0 + ch * P])
            t1s = sb.tile([NF, CH * P], F32, tag="p2t1s")
            nc.vector.tensor_scalar(out=t1s[:, :ch * P], in0=ttT_slab[:, :ch * P],
                                    scalar1=scale_col[:], scalar2=None,
                                    op0=OP.mult)
            nc.vector.tensor_scalar(out=t1s[:, :ch * P], in0=t1s[:, :ch * P],
                                    scalar1=shift_col[:], scalar2=None,
                                    op0=OP.add)
            e2 = sb.tile([NF, CH * P], F32, tag="p2e2")
            nc.scalar.activation(e2[:, :ch * P], t1s[:, :ch * P], AF.Exp)
            sp = sb.tile([NF, CH * P], F32, tag="p2sp")
            nc.scalar.activation(sp[:, :ch * P], e2[:, :ch * P], AF.Ln, bias=1.0)
            yslab = sb.tile([P, CH, NF], F32, tag="yslab")
            ysrc = bass.AP(y_own.ap().tensor, e0 * NF,
                           [[NF, P], [P * NF, ch], [1, NF]])
            nc.sync.dma_start(yslab[:, :ch, :], ysrc)
            for t in range(ch):
                es = t * P
                yp = ps.tile([P, NF], F32, space="PSUM", tag="mm")
                nc.tensor.matmul(out=yp[:], lhsT=sp[:, es:es + P],
                                 rhs=ident[:NF, :NF], start=True, stop=True)
                y2t = sb.tile([P, NF], F32, tag="y2t")
                nc.vector.tensor_tensor(out=y2t[:], in0=yp[:], in1=yslab[:, t, :],
                                        op=OP.add)
                nc.sync.dma_start(y2m[e0 + es:e0 + es + P, :], y2t[:])

        # ---------------- AllGather y2
        if sim_mode:
            for kk in range(NC):
                nc.sync.dma_start(y2tab[kk * L1:(kk + 1) * L1, :], y2m[:])
        else:
            nc.gpsimd.collective_compute(
                "AllGather", OP.bypass, replica_groups=[list(range(NC))],
                ins=[y2m[:].opt()], outs=[y2tab[:].opt()])

        # ---------------- phase 3
        edge_phase(m3, g3_src, g3_dst, g3_sidx, zT_s, drel3, mpre3,
                   y2tab, y2m, wsd2_s, we2_s, "bnm2_g", "bnm2_b",
                   htab3, LG, store_mT=False)
        node_epilogue(htab3, m3, L1, E, "bn2_g", "bn2_b", 256,
                      y2m, yout)

    nc.compile()
    return nc


# =============================================================== entry point

_CACHE = {}


def kernel(x, y, z, params, g_src, g_dst, lg_src, lg_dst, cfg=None):
    cfg = cfg or CFG
    meta, in_maps, post = prep(x, y, z, params, g_src, g_dst, lg_src, lg_dst, cfg)
    key = (meta["L1"], meta["meta3"]["L"],
           tuple((c["t0"], c["nt"], c["sbase"], c["dbase"]) for c in meta["meta1"]["calls"]),
           tuple((c["t0"], c["nt"], c["sbase"], c["dbase"]) for c in meta["meta3"]["calls"]))
    if key not in _CACHE:
        _CACHE.clear()
        _CACHE[key] = build(meta)
    nc = _CACHE[key]
    res = bass_utils.run_bass_kernel_spmd(nc, in_maps, core_ids=list(range(NC)))

    N, E = cfg["n_nodes"], cfg["n_edges"]
    npc, L1 = meta["npc"], meta["L1"]
    x_out = np.concatenate([res.results[k]["xout"] for k in range(NC)], 0)
    y_out = np.zeros((E, NF), np.float32)
    for k in range(NC):
        st = post["cores1"][k]["stream"]
        r = post["cores1"][k]["real"]
        orig = post["pcs"][k][2][st[r]]
        y_out[orig] = res.results[k]["yout"][r]
    return x_out, y_out


def np_reference(x, y, z, params, g_src, g_dst, lg_src, lg_dst):
    """Pure-numpy mirror of reference.reference (f64 for stability)."""
    def tonp(t):
        return np.asarray(t, np.float64)
    def bn(a, g, b):
        mu = a.mean(0)
        var = a.var(0)
        return (a - mu) / np.sqrt(var + EPS) * tonp(g) + tonp(b)
    def softplus(a):
        return np.log1p(np.exp(-np.abs(a))) + np.maximum(a, 0)
    def sigmoid(a):
        return 1.0 / (1.0 + np.exp(-a))
    def cgcnn(src, dst, nf, ef, p, n):
        m = (tonp(nf) @ tonp(p["W_src"]) + tonp(p["b_src"]))[src] \
            + (tonp(nf) @ tonp(p["W_dst"]) + tonp(p["b_dst"]))[dst] \
            + tonp(ef) @ tonp(p["W_edge"]) + tonp(p["b_edge"])
        m = bn(m, p["bn_m_g"], p["bn_m_b"])
        hf, hs = m[:, :m.shape[1] // 2], m[:, m.shape[1] // 2:]
        m = sigmoid(hf) * softplus(hs)
        h = np.zeros((n, m.shape[1]))
        np.add.at(h, dst, m)
        h = bn(h, p["bn_g"], p["bn_b"])
        return softplus(tonp(nf) + h), m
    x_out, m = cgcnn(g_src, g_dst, x, y, params["node"], x.shape[0])
    mb = softplus(bn(np.concatenate([tonp(y), m], 1) @ tonp(params["Wb"]) + tonp(params["bb"]),
                     params["bn_b_g"], params["bn_b_b"]))
    y2 = tonp(y) + mb
    y_out, _ = cgcnn(lg_src, lg_dst, y2, z, params["edge"], y.shape[0])
    return x_out.astype(np.float32), y_out.astype(np.float32)


def _np_params(rng, nf, ef):
    return {
        "W_src": (0.05 * rng.normal(size=(nf, 2 * nf))).astype(np.float32),
        "b_src": np.zeros(2 * nf, np.float32),
        "W_dst": (0.05 * rng.normal(size=(nf, 2 * nf))).astype(np.float32),
        "b_dst": np.zeros(2 * nf, np.float32),
        "W_edge": (0.05 * rng.normal(size=(ef, 2 * nf))).astype(np.float32),
        "b_edge": np.zeros(2 * nf, np.float32),
        "bn_m_g": (1 + 0.1 * rng.normal(size=2 * nf)).astype(np.float32),
        "bn_m_b": (0.1 * rng.normal(size=2 * nf)).astype(np.float32),
        "bn_g": (1 + 0.1 * rng.normal(size=nf)).astype(np.float32),
        "bn_b": (0.1 * rng.normal(size=nf)).astype(np.float32),
    }


if __name__ == "__main__":
    mini = dict(n_nodes=1024, n_edges=8192, n_lg_edges=16384)
    rng = np.random.default_rng(0)
    x = rng.normal(size=(mini["n_nodes"], NF)).astype(np.float32)
    y = rng.normal(size=(mini["n_edges"], NF)).astype(np.float32)
    z = rng.normal(size=(mini["n_lg_edges"], NF)).astype(np.float32)
    gs = rng.integers(0, mini["n_nodes"], mini["n_edges"]).astype(np.int32)
    gd = rng.integers(0, mini["n_nodes"], mini["n_edges"]).astype(np.int32)
    ls = rng.integers(0, mini["n_edges"], mini["n_lg_edges"]).astype(np.int32)
    ld = rng.integers(0, mini["n_edges"], mini["n_lg_edges"]).astype(np.int32)
    params = {
        "node": _np_params(rng, NF, NF),
        "edge": _np_params(rng, NF, NF),
        "Wb": (0.05 * rng.normal(size=(2 * NF, NF))).astype(np.float32),
        "bb": np.zeros(NF, np.float32),
        "bn_b_g": (1 + 0.1 * rng.normal(size=NF)).astype(np.float32),
        "bn_b_b": (0.1 * rng.normal(size=NF)).astype(np.float32),
    }
    exp_x, exp_y = np_reference(x, y, z, params, gs, gd, ls, ld)
    got_x, got_y = kernel(x, y, z, params, gs, gd, ls, ld, cfg=mini)
    rx = np.abs(got_x - exp_x).max() / (np.abs(exp_x).max() + 1e-9)
    ry = np.abs(got_y - exp_y).max() / (np.abs(exp_y).max() + 1e-9)
    print(f"mini rel err x: {rx:.3e}  y: {ry:.3e}")
    print("MINI", "PASSED" if max(rx, ry) < 2e-2 else "FAILED")
